# revision 21
# baseline (speedup 1.0000x reference)
"""BitNet linear (y = (x @ sign(W).T + b) * mean(|W|)) on 8 trn2 NeuronCores.

Sharding: column-parallel — W is sharded along out_features across the 8
cores, x is replicated, each core produces out[:, shard] and the host
concatenates.

Device algorithm (per core):
  1. DMA the core's W^T shard (f32, chunks alternating across the two HW DGE
     queues so W streaming overlaps the x DMAs), compute w_q = sign(W) in
     bf16 via a mult/min/max clamp (exact {-1,0,+1}) into per-chunk tiles
     (block-0 matmuls gate only on their own chunk), and per-chunk |W| sums
     on ScalarE.
  2. alpha = mean|W| entirely off the PE stream: ScalarE accum -> AllReduce
     -> partition-broadcast via a DRAM round-trip on the scalar DMA queue.
     The raw bias is ones-matmul-broadcast at the head of the PE stream (no
     alpha dependency) and alpha-scaled later on ScalarE, so the PE never
     waits on the collective.
  3. Main matmul: a SINGLE bf16 pass (x rounded to bf16): w_q is exact in
     bf16 and the b*s*i contraction of 4096 bf16 roundoffs lands at ~1.7e-3
     rel L2 — well under the 2e-2 gate — at 1 PE cycle/row vs 2 for the old
     hi+lo split and 4 for fp32.  x is host-pre-tiled to [KK, MB, 128, KO,
     128] so every stage-DMA partition line is one contiguous 2KB read.
     bias*alpha is fused into the PSUM->SBUF scale (scalar_tensor_tensor:
     out = psum*alpha + bias*alpha); the next block's x DMA+convert is
     emitted ahead of the copies so the in-order DVE stream keeps the PE fed
     across block boundaries.
  Post-scheduling, consolidate_pe_sem_incs() strips the per-matmul engine-
  clock semaphore increments down to the ~3% that wait thresholds actually
  reference (~26ns serialized EVT_SEM write each, ~0.2ms/rep saved).

Measured (reps-difference timing, 8 cores): 3707781 ns (hi+lo baseline)
-> 2026590 ns (single bf16 pass) -> 1867123 ns (sem-inc consolidation);
single-pass PE roofline is ~1768 us.  Relative error 1.66e-3.
"""

import numpy as np

import concourse.bass as bass
import concourse.mybir as mybir
import concourse.tile as tile
from concourse.bass import ds
from concourse.vector_clock import ScopedClock

# ---------------------------------------------------------------------------
# Compatibility patch: the pinned walrus (neuronxcc) in this container only
# supports ONE ge-wait per instruction and no eq-waits; the concourse Tile
# tail emits a Drain with multiple waits plus an eq-wait barrier butterfly
# ("Too many sync wait commands").  Replace the tail with one-wait-per-nop
# splitting and the NRT-expanded PSEUDO_SYNC_BARRIER (the pre-butterfly
# mechanism this walrus/NRT pair supports).
# ---------------------------------------------------------------------------


def _compat_drain_and_barrier(self, tick_clock, wait_clock):
    nc = self.nc
    coll = nc.sync.nop(nofuse=True)
    wait_clock.add_sem_waits(coll.ins, ScopedClock({None: tick_clock.global_clock}))
    si = coll.ins.sync_info
    if si is not None:
        waits = list(si.on_wait)
        if len(waits) > 1:
            coll.ins.sync_info = mybir.SyncInfo(
                on_wait=[waits[0]], on_update=list(si.on_update)
            )
            for w in waits[1:]:
                extra = nc.sync.nop(nofuse=True)
                extra.ins.sync_info = mybir.SyncInfo(on_wait=[w], on_update=[])
    for eng in nc.engines.values():
        eng.drain()
    nc._nrt_pseudo_barrier()
    popped = nc._tile_sem_poison_stack.pop()
    assert popped is self._sem_poison
    nc.clear_and_free_semaphores(list(self.sems.allocated().values()))
    nc._nrt_pseudo_barrier()


tile.TileContext._drain_and_barrier = _compat_drain_and_barrier


# NOTE: walrus's own LDWEIGHTS optimizer (--enable-ldw-opt=true) was tried
# via a run_command patch and the walrus_driver crashes on this program —
# the flag is hardcoded off in bass_utils for a reason.  Do not re-enable.

_legalize_ctr = [0]


def legalize_waits(nc):
    """Split instructions carrying more than the HW-supported number of sem
    waits (1; EventSemaphore: 2) into preceding one-wait NoOps on the same
    engine — semantically identical, encodable by the pinned walrus."""
    import bass_rust

    for f in nc.m.functions:
        for bb in f.blocks:
            il = bb.instructions
            i = 0
            while i < len(il):
                ins = il[i]
                si = ins.sync_info
                waits = list(si.on_wait) if si is not None else []
                limit = 2 if type(ins).__name__ == "InstEventSemaphore" else 1
                if len(waits) > limit:
                    keep = waits[-limit:]
                    spill = waits[:-limit]
                    for w in spill:
                        _legalize_ctr[0] += 1
                        nop = bass_rust.InstNoOp(
                            name=f"I-lw{_legalize_ctr[0]}", ins=[], outs=[]
                        )
                        nop.engine = ins.engine
                        nop.sync_info = mybir.SyncInfo(on_wait=[w], on_update=[])
                        il.insert(i, nop)
                        i += 1
                    ins.sync_info = mybir.SyncInfo(
                        on_wait=keep, on_update=list(si.on_update)
                    )
                i += 1


def consolidate_pe_sem_incs(nc):
    """Merge per-matmul +1 engine-clock increments into value-k increments.

    Every InstMatmult carries a sem-inc(+1) on the PE vector-clock semaphore;
    each inc is a serialized EVT_SEM register write (~26ns — sem-tail model in
    the tensor-engine guide), ~8k of which cost ~0.2ms/rep.  Matmuls complete
    in pc order (HW-verified, Δend=0), so the t-th increment completes at the
    t-th matmul.  Keep an increment only at instructions whose cumulative
    count equals some wait threshold actually referenced on that semaphore
    (update_value = count delta since the previous kept inc) plus a final
    top-up — every wait is then satisfied at the exact same instruction as
    before, with ~97% fewer EVT_SEM writes."""
    import bass_rust

    il = [
        ins
        for func in nc.m.functions
        for bb in func.blocks
        for ins in bb.instructions
    ]
    PE = mybir.EngineType.PE

    # collect every wait threshold on each sem that the PE increments
    pe_incs = {}  # sem id -> list of instructions carrying +1 incs, in order
    for ins in il:
        si = ins.sync_info
        if si is None:
            continue
        for u in list(si.on_update):
            if u.sync_type != "semaphore" or u.update_mode != "sem-inc":
                continue
            if ins.engine != PE:
                continue
            pe_incs.setdefault(u.id, []).append(ins)
    for sem, incs in pe_incs.items():
        # bail if any non-PE engine also updates this sem, any inc isn't +1,
        # or any wait on it is register-based
        ok = True
        thresholds = set()
        for ins in il:
            si = ins.sync_info
            if si is None:
                continue
            for u in list(si.on_update):
                if u.id == sem and ins.engine != PE:
                    ok = False
            for w in list(si.on_wait):
                if w.sync_type == "semaphore" and w.id == sem:
                    if w.wait_value is None or w.wait_reg:
                        ok = False
                    else:
                        thresholds.add(w.wait_value)
        if not ok:
            continue
        if any(
            u.update_value not in (None, 1)
            for ins in incs
            for u in list(ins.sync_info.on_update)
            if u.id == sem
        ):
            continue
        total = len(incs)
        keep = sorted(t for t in thresholds if 0 < t <= total)
        if total not in keep:
            keep.append(total)
        # this walrus asserts UpdateValue == 1, so instead of value-k incs we
        # keep a +1 inc only at each threshold-crossing instruction and
        # renumber every wait to its threshold's rank — each wait is then
        # satisfied by the exact same instruction as before
        rank = {t: i + 1 for i, t in enumerate(keep)}
        keep_set = set(keep)
        count = 0
        for ins in incs:
            count += 1
            si = ins.sync_info
            if count not in keep_set:
                ins.sync_info = mybir.SyncInfo(
                    on_wait=list(si.on_wait),
                    on_update=[u for u in list(si.on_update) if u.id != sem],
                )
        for ins in il:
            si = ins.sync_info
            if si is None:
                continue
            changed = False
            waits = list(si.on_wait)
            for w in waits:
                if (
                    w.sync_type == "semaphore"
                    and w.id == sem
                    and w.wait_value is not None
                    and 0 < w.wait_value <= total
                ):
                    w.wait_value = rank[w.wait_value]
                    changed = True
            if changed:
                ins.sync_info = mybir.SyncInfo(
                    on_wait=waits, on_update=list(si.on_update)
                )


def elide_redundant_ldweights(nc):
    """Drop InstLdweights that reload the exact weights already sitting in
    the PE array.  bass lowers every InstMatmult to an Ldweights+Matmult
    pair; consecutive matmuls sharing one stationary tile reload it each
    time (~107ns of PE time apiece).  Two Ldweights with no other Ldweights
    between them and the same (tile name, offset, pattern) provably load
    identical content — tile names are unique per pool.tile() call and each
    tile is written before its first consumer only.  Elided instructions
    carrying semaphore waits/updates become NoOps to preserve sync."""
    import bass_rust

    n_elided = 0
    for f in nc.m.functions:
        for bb in f.blocks:
            il = bb.instructions
            last_key = None
            for i in range(len(il)):
                ins = il[i]
                nm = type(ins).__name__
                if nm != "InstLdweights":
                    continue
                a = ins.ins[0]
                bap = getattr(a, "bass_ap", None)
                if bap is None:
                    last_key = None
                    continue
                key = (
                    bap.tensor.name,
                    bap.offset,
                    str(bap.ap),
                    ins.perf_mode,
                    ins.is_transpose,
                    ins.tile_position,
                )
                if key == last_key:
                    si = ins.sync_info
                    has_sync = si is not None and (
                        list(si.on_wait) or list(si.on_update)
                    )
                    nop = bass_rust.InstNoOp(name=f"{ins.name}-eld", ins=[], outs=[])
                    nop.engine = ins.engine
                    if has_sync:
                        nop.sync_info = mybir.SyncInfo(
                            on_wait=list(si.on_wait), on_update=list(si.on_update)
                        )
                    il[i] = nop
                    n_elided += 1
                else:
                    last_key = key
    return n_elided


F32 = mybir.dt.float32
BF16 = mybir.dt.bfloat16

P = 128  # partitions


def prep_x(x2: np.ndarray) -> np.ndarray:
    """Host-side tiling of x [M, K] into [KK, MB, P, KO, P] so each SBUF
    partition's stage-tile slice is one contiguous 2KB DMA line.
    xR[kk, mb, p, ko, j] = x2[mb*128 + j, kk*(KO*128) + ko*128 + p]."""
    M, K = x2.shape
    KO = min(4, K // P)
    xr = x2.reshape(M // P, P, K // (P * KO), KO, P).transpose(2, 0, 4, 3, 1)
    return np.ascontiguousarray(xr)


def build_bitnet_nc(
    M: int,
    K: int,
    N_shard: int,
    n_total_weight: int,
    n_cores: int = 8,
    nsplits: int = 1,
    debug: bool = False,
    legalize: bool = True,
    reps: int = 1,
    skip_cc: bool = False,
    pipeline_splits: bool = True,
    fuse_bias: bool = True,
    elide_ldweights: bool = False,
):
    """Build the per-core Bass program.

    M: rows of x (B*S), K: in_features, N_shard: out_features per core.
    n_total_weight: total element count of the full W (for mean(|W|)).
    """
    assert M % P == 0 and K % P == 0
    K_CHUNKS = K // P
    KO = min(4, K_CHUNKS)  # k-chunks fetched per x DMA
    assert K_CHUNKS % KO == 0
    KK = K_CHUNKS // KO
    N_TILE = min(512, N_shard)
    assert N_shard % N_TILE == 0
    NB = N_shard // N_TILE
    M_BLOCKS = M // P

    nc = bass.Bass(num_devices=n_cores)
    # x pre-tiled on host to [KK, MB, P, KO, P]: partition p of stage tile
    # (kk, m) reads one contiguous KO*128*4B = 2KB DMA line instead of KO
    # scattered 512B lines
    xT = nc.declare_dram_parameter(
        "xT", [KK, M_BLOCKS, P, KO, P], F32, isOutput=False
    )
    wT = nc.declare_dram_parameter("wT", [K, N_shard], F32, isOutput=False)
    bias_d = nc.declare_dram_parameter("bias", [N_shard], F32, isOutput=False)
    out_d = nc.declare_dram_parameter("out", [M, N_shard], F32, isOutput=True)

    with tile.TileContext(nc) as tc:
        wq_pool = tc.tile_pool(name="wq", bufs=1)
        wstage = tc.tile_pool(name="wstage", bufs=3)
        small = tc.tile_pool(name="small", bufs=1)
        xstage = tc.tile_pool(name="xstage", bufs=2)
        xhi_pool = tc.tile_pool(name="xhi", bufs=2 * KK - 1)
        xlo_pool = tc.tile_pool(name="xlo", bufs=2 * KK - 1)
        out_pool = tc.tile_pool(name="outp", bufs=2)
        absd_pool = tc.tile_pool(name="absd", bufs=1)
        psum_pool = tc.tile_pool(name="psum", bufs=2, space="PSUM")
        dram = tc.tile_pool(name="dram", bufs=1, space="DRAM")

        with (
            wq_pool as wq_p,
            wstage as wst_p,
            small as small_p,
            xstage as xst_p,
            xhi_pool as xhi_p,
            xlo_pool as xlo_p,
            out_pool as out_p,
            absd_pool as absd_p,
            psum_pool as ps_p,
            dram as dram_p,
        ):
            # ---------------- head: raw-bias broadcast (PE, no alpha dep) ---
            # PE's in-order stream must never wait on the AllReduce-produced
            # alpha, so broadcast the RAW bias first (ones-matmul) and scale
            # the broadcast by alpha later on ScalarE.
            bias_sb = small_p.tile([1, N_shard], F32)
            nc.sync.dma_start(bias_sb[:], bias_d[None, :])
            ones_row = small_p.tile([1, P], F32)
            nc.vector.memset(ones_row[:], 1.0)
            onesf = small_p.tile([1, P], F32)
            nc.vector.memset(onesf[:], 1.0 / float(n_total_weight))
            bias_bc = small_p.tile([P, N_shard], BF16)
            if fuse_bias:
                for n in range(NB):
                    bps = ps_p.tile([P, N_TILE], F32, tag="ps", name=f"bps{n}")
                    nc.tensor.matmul(
                        bps[:],
                        ones_row[:],
                        bias_sb[:, ds(n * N_TILE, N_TILE)],
                        start=True,
                        stop=True,
                    )
                    nc.vector.tensor_copy(bias_bc[:, ds(n * N_TILE, N_TILE)], bps[:])

            # x block 0 queued on the sync DMA queue ahead of the W chunks
            def emit_split(m, tag):
                his, los = [], []
                for kk in range(KK):
                    xs = xst_p.tile([P, KO, P], F32, tag="xs", name=f"xs{tag}_{kk}")
                    nc.sync.dma_start(xs[:], xT[kk, m])
                    hi = xhi_p.tile(
                        [P, KO, P], BF16, tag="xhi", name=f"hi{tag}_{kk}"
                    )
                    nc.vector.tensor_copy(hi[:], xs[:])
                    his.append(hi)
                    if nsplits == 2:
                        lo = xlo_p.tile(
                            [P, KO, P], BF16, tag="xlo", name=f"lo{tag}_{kk}"
                        )
                        nc.vector.tensor_sub(lo[:], xs[:], hi[:])
                        los.append(lo)
                return his, los

            pending = emit_split(0, "b0") if pipeline_splits else None

            # ---------------- Phase A: sign(W) + |W| partial sums ----------
            # W chunks alternate between the two HW DGE queues (sync/scalar)
            # so W streaming overlaps the x-block DMAs; each chunk gets its
            # own wq tile so block-0 matmuls gate only on their own chunk.
            wq_tiles = []
            acc = small_p.tile([P, K_CHUNKS], F32)
            abs_dump = absd_p.tile([P, N_shard], BF16)
            for k in range(K_CHUNKS):
                wst = wst_p.tile([P, N_shard], F32, tag="wst")
                dma_eng = nc.scalar if (k % 2) else nc.sync
                dma_eng.dma_start(wst[:], wT[k * P : (k + 1) * P, :])
                # per-chunk |W| sum on ScalarE: activation(Abs) accumulates
                # the row sum into acc while DVE does the sign clamp
                nc.scalar.activation(
                    abs_dump[:],
                    wst[:],
                    mybir.ActivationFunctionType.Abs,
                    accum_out=acc[:, k : k + 1],
                )
                # sign via clamp: s = max(min(w * 1e30, 1), -1), exact
                # {-1, 0, +1} (|w| > 1e-30 or w == 0 for any normal float);
                # second op runs in place on the wq slice
                wqk = wq_p.tile([P, N_shard], BF16, name=f"wq{k}")
                nc.vector.tensor_scalar(
                    wqk[:],
                    wst[:],
                    1e30,
                    1.0,
                    mybir.AluOpType.mult,
                    mybir.AluOpType.min,
                )
                nc.vector.tensor_scalar(
                    wqk[:], wqk[:], -1.0, None, mybir.AluOpType.max
                )
                wq_tiles.append(wqk)

            # ---------------- Phase B: alpha = mean|W|, entirely off-PE ----
            # Partition-reduce + broadcast via a DRAM round-trip on the
            # scalar DMA queue (head-of-line safe: nothing else uses it
            # after the W odds) + ScalarE accum ops.  The PE stream never
            # sees any of this.
            acc_dump = small_p.tile([P, K_CHUNKS], F32)
            asum = small_p.tile([P, 1], F32)
            nc.scalar.activation(
                acc_dump[:],
                acc[:],
                mybir.ActivationFunctionType.Copy,
                accum_out=asum[:],
            )
            cc_in = dram_p.tile([P, 1], F32)
            cc_out = dram_p.tile(
                [P, 1], F32, addr_space="Shared" if n_cores > 4 else "Local"
            )
            nc.scalar.dma_start(cc_in[:], asum[:])
            if skip_cc:
                nc.scalar.dma_start(cc_out[:], cc_in[:])
            else:
                nc.gpsimd.collective_compute(
                    "AllReduce",
                    mybir.AluOpType.add,
                    replica_groups=[list(range(n_cores))],
                    ins=[cc_in.opt()],
                    outs=[cc_out.opt()],
                )
            arow = small_p.tile([1, P], F32)
            nc.scalar.dma_start(arow[:], cc_out[:, 0][None, :])
            sdump = small_p.tile([1, P], F32)
            stot = small_p.tile([1, 1], F32)
            nc.scalar.activation(
                sdump[:],
                arow[:],
                mybir.ActivationFunctionType.Copy,
                accum_out=stot[:],
            )
            # alpha broadcast along free dim: srow = onesf * stot (per-
            # partition scalar), then DRAM round-trip to [P, 1] layout
            srow = small_p.tile([1, P], F32)
            nc.scalar.activation(
                srow[:],
                onesf[:],
                mybir.ActivationFunctionType.Copy,
                scale=stot[:1, :],
            )
            ad = dram_p.tile([P, 1], F32)
            nc.scalar.dma_start(ad[:, 0][None, :], srow[:])
            alpha = small_p.tile([P, 1], F32)
            nc.scalar.dma_start(alpha[:], ad[:])
            if fuse_bias:
                # scale the raw-bias broadcast by alpha in place (ScalarE)
                nc.scalar.activation(
                    bias_bc[:],
                    bias_bc[:],
                    mybir.ActivationFunctionType.Copy,
                    scale=alpha[:],
                )

            # ---------------- Phase C: main matmul -------------------------
            # Software-pipelined: block b+1's x DMA + hi/lo split is emitted
            # BEFORE block b's PSUM->SBUF copies, so the in-order DVE stream
            # produces the next block's stationaries while the PE runs block
            # b's matmuls (otherwise DVE blocks on the copy's PSUM wait and
            # the PE idles ~3us per block boundary — enough to re-throttle
            # the HAM clock gate).
            total_blocks = reps * M_BLOCKS

            for bi in range(total_blocks):
                m = bi % M_BLOCKS
                if pipeline_splits:
                    his, los = pending
                else:
                    his, los = emit_split(m, f"b{bi}")

                psums = [
                    ps_p.tile([P, N_TILE], F32, tag="ps", name=f"ps{n}")
                    for n in range(NB)
                ]
                first = True
                for kk in range(KK):
                    for ko in range(KO):
                        k = kk * KO + ko
                        parts = [his[kk][:, ko, :]]
                        if nsplits == 2:
                            parts.append(los[kk][:, ko, :])
                        last = kk == KK - 1 and ko == KO - 1
                        for pi, lhsT in enumerate(parts):
                            stop_here = fuse_bias and last and pi == len(parts) - 1
                            for n in range(NB):
                                nc.tensor.matmul(
                                    psums[n][:],
                                    lhsT,
                                    wq_tiles[k][:, ds(n * N_TILE, N_TILE)],
                                    start=first,
                                    stop=stop_here,
                                )
                            first = False
                if not fuse_bias:
                    # bias via K=1 ones-matmul (also closes the accumulation)
                    for n in range(NB):
                        nc.tensor.matmul(
                            psums[n][:],
                            ones_row[:],
                            bias_sb[:, ds(n * N_TILE, N_TILE)],
                            start=False,
                            stop=True,
                        )

                if pipeline_splits and bi + 1 < total_blocks:
                    pending = emit_split((bi + 1) % M_BLOCKS, f"b{bi + 1}")

                osb = out_p.tile([P, N_shard], F32, tag="osb")
                for n in range(NB):
                    if fuse_bias:
                        nc.vector.scalar_tensor_tensor(
                            osb[:, ds(n * N_TILE, N_TILE)],
                            psums[n][:],
                            alpha[:],
                            bias_bc[:, ds(n * N_TILE, N_TILE)],
                            mybir.AluOpType.mult,
                            mybir.AluOpType.add,
                        )
                    else:
                        nc.vector.tensor_scalar_mul(
                            osb[:, ds(n * N_TILE, N_TILE)], psums[n][:], alpha[:]
                        )
                nc.sync.dma_start(out_d[m * P : (m + 1) * P, :], osb[:])

    if elide_ldweights:
        # NOTE: measured SLOWER on HW (+~570us/rep at reps=4): walrus appears
        # to re-emit the weight load per orphan Matmult in a self-loading,
        # non-pipelined form, losing the LDWEIGHTS/MATMUL overlap.
        elide_redundant_ldweights(nc)
    consolidate_pe_sem_incs(nc)
    if legalize:
        legalize_waits(nc)  # required for walrus; CoreSim chokes on raw NoOps
    return nc


def run_bitnet(
    x: np.ndarray,
    weight: np.ndarray,
    bias: np.ndarray,
    n_cores: int = 8,
    nsplits: int = 1,
    trace: bool = False,
):
    """Host driver: shard, run on n_cores, gather. x: [..., K], weight: [N, K]."""
    from concourse.bass_utils import run_bass_kernel_spmd

    lead_shape = x.shape[:-1]
    K = x.shape[-1]
    N = weight.shape[0]
    M = int(np.prod(lead_shape))
    assert weight.shape == (N, K) and bias.shape == (N,)
    assert N % n_cores == 0
    N_shard = N // n_cores

    x2 = np.ascontiguousarray(x.reshape(M, K).astype(np.float32, copy=False))
    xT = prep_x(x2)
    w = weight.astype(np.float32, copy=False)

    nc = build_bitnet_nc(M, K, N_shard, N * K, n_cores=n_cores, nsplits=nsplits)

    in_maps = []
    for c in range(n_cores):
        wTc = np.ascontiguousarray(w[c * N_shard : (c + 1) * N_shard, :].T)
        bc = np.ascontiguousarray(bias[c * N_shard : (c + 1) * N_shard]).astype(
            np.float32, copy=False
        )
        in_maps.append({"xT": xT, "wT": wTc, "bias": bc})

    res = run_bass_kernel_spmd(
        nc, in_maps, core_ids=list(range(n_cores)), trace=trace
    )
    out = np.empty((M, N), dtype=np.float32)
    for c in range(n_cores):
        out[:, c * N_shard : (c + 1) * N_shard] = res.results[c]["out"]
    return out.reshape(*lead_shape, N), res


_RUNNER_CACHE: dict = {}


def _cached_pjrt_run(M, K, N_shard, n_cores, in_maps):
    """Compile-once-per-shape PJRT executor (same machinery as
    run_bitnet_timed, which is HW-validated); repeat kernel() calls skip the
    multi-minute NEFF rebuild and only pay transfer + execution."""
    import jax
    import jax.numpy as jnp
    from jax.sharding import Mesh, NamedSharding, PartitionSpec
    from jax.experimental.shard_map import shard_map

    from concourse import bass2jax
    from concourse.bass2jax import _bass_exec_p, partition_id_tensor

    key = (M, K, N_shard, n_cores)
    if key not in _RUNNER_CACHE:
        bass2jax.install_neuronx_cc_hook()
        nc = build_bitnet_nc(M, K, N_shard, N_shard * n_cores * K, n_cores=n_cores)
        partition_name = (
            nc.partition_id_tensor.name if nc.partition_id_tensor else None
        )
        in_names, out_names, out_avals, zero_outs = [], [], [], []
        for alloc in nc.m.functions[0].allocations:
            if not isinstance(alloc, mybir.MemoryLocationSet):
                continue
            name = alloc.memorylocations[0].name
            if alloc.kind == "ExternalInput":
                if name != partition_name:
                    in_names.append(name)
            elif alloc.kind == "ExternalOutput":
                shape = tuple(alloc.tensor_shape)
                dtype = mybir.dt.np(alloc.dtype)
                out_names.append(name)
                out_avals.append(jax.core.ShapedArray(shape, dtype))
                zero_outs.append(np.zeros(shape, dtype))
        n_params = len(in_names)
        n_outs = len(out_avals)
        param_names = list(in_names)
        in_names = in_names + out_names
        if partition_name is not None:
            in_names.append(partition_name)
        donate = tuple(range(n_params, n_params + n_outs))

        def _body(*args):
            operands = list(args)
            if partition_name is not None:
                operands.append(partition_id_tensor())
            return tuple(
                _bass_exec_p.bind(
                    *operands,
                    out_avals=tuple(out_avals),
                    in_names=tuple(in_names),
                    out_names=tuple(out_names),
                    lowering_input_output_aliases=(),
                    sim_require_finite=True,
                    sim_require_nnan=True,
                    nc=nc,
                )
            )

        devices = jax.devices()[:n_cores]
        mesh = Mesh(np.asarray(devices), ("core",))
        sh = NamedSharding(mesh, PartitionSpec("core"))
        sharded = jax.jit(
            shard_map(
                _body,
                mesh=mesh,
                in_specs=(PartitionSpec("core"),) * (n_params + n_outs),
                out_specs=(PartitionSpec("core"),) * len(out_names),
                check_rep=False,
            ),
            donate_argnums=donate,
            keep_unused=True,
        )
        zfns = [
            jax.jit(
                lambda shp=(n_cores * z.shape[0], *z.shape[1:]),
                dt=z.dtype: jnp.zeros(shp, dt),
                out_shardings=sh,
            )
            for z in zero_outs
        ]
        _RUNNER_CACHE[key] = (sharded, param_names, out_names, out_avals, sh, zfns)

    sharded, param_names, out_names, out_avals, sh, zfns = _RUNNER_CACHE[key]
    import jax

    concat_in = [
        jax.device_put(
            np.concatenate(
                [np.asarray(in_maps[c][nm]) for c in range(n_cores)], 0
            ),
            sh,
        )
        for nm in param_names
    ]
    out_arrs = sharded(*concat_in, *[f() for f in zfns])
    oi = out_names.index("out")
    glob = np.asarray(out_arrs[oi]).reshape(n_cores, *out_avals[oi].shape)
    return [glob[c] for c in range(n_cores)]


def kernel(x: np.ndarray, weight: np.ndarray, bias: np.ndarray) -> np.ndarray:
    lead_shape = x.shape[:-1]
    K = x.shape[-1]
    N = weight.shape[0]
    M = int(np.prod(lead_shape))
    n_cores = 8
    N_shard = N // n_cores

    x2 = np.ascontiguousarray(x.reshape(M, K).astype(np.float32, copy=False))
    xT = prep_x(x2)
    w = weight.astype(np.float32, copy=False)
    in_maps = []
    for c in range(n_cores):
        in_maps.append(
            {
                "xT": xT,
                "wT": np.ascontiguousarray(w[c * N_shard : (c + 1) * N_shard, :].T),
                "bias": np.ascontiguousarray(
                    bias[c * N_shard : (c + 1) * N_shard]
                ).astype(np.float32, copy=False),
            }
        )
    shards = _cached_pjrt_run(M, K, N_shard, n_cores, in_maps)
    out = np.empty((M, N), dtype=np.float32)
    for c in range(n_cores):
        out[:, c * N_shard : (c + 1) * N_shard] = shards[c]
    return out.reshape(*lead_shape, N)


def run_bitnet_timed(
    x: np.ndarray,
    weight: np.ndarray,
    bias: np.ndarray,
    n_cores: int = 8,
    nsplits: int = 1,
    reps: int = 4,
    rounds: int = 6,
):
    """Like run_bitnet, but measures HW time via the reps-difference method:
    build the kernel once plain and once with the main loop unrolled `reps`
    times, time single dispatches of each (min over `rounds`), and divide the
    delta by reps-1.  This cancels the multi-ms, noisy axon dispatch floor.
    Returns (out, per_exec_seconds, diag)."""
    import time

    import jax
    import jax.numpy as jnp
    from jax.sharding import Mesh, NamedSharding, PartitionSpec
    from jax.experimental.shard_map import shard_map

    from concourse import bass2jax
    from concourse.bass2jax import _bass_exec_p, partition_id_tensor

    lead_shape = x.shape[:-1]
    K = x.shape[-1]
    N = weight.shape[0]
    M = int(np.prod(lead_shape))
    N_shard = N // n_cores

    x2 = np.ascontiguousarray(x.reshape(M, K).astype(np.float32, copy=False))
    xT = prep_x(x2)
    w = weight.astype(np.float32, copy=False)

    bass2jax.install_neuronx_cc_hook()

    in_maps = []
    for c in range(n_cores):
        wTc = np.ascontiguousarray(w[c * N_shard : (c + 1) * N_shard, :].T)
        bc = np.ascontiguousarray(bias[c * N_shard : (c + 1) * N_shard]).astype(
            np.float32, copy=False
        )
        in_maps.append({"xT": xT, "wT": wTc, "bias": bc})

    devices = jax.devices()[:n_cores]
    mesh = Mesh(np.asarray(devices), ("core",))
    sh = NamedSharding(mesh, PartitionSpec("core"))

    def make_runner(nc):
        partition_name = (
            nc.partition_id_tensor.name if nc.partition_id_tensor else None
        )
        in_names, out_names, out_avals, zero_outs = [], [], [], []
        for alloc in nc.m.functions[0].allocations:
            if not isinstance(alloc, mybir.MemoryLocationSet):
                continue
            name = alloc.memorylocations[0].name
            if alloc.kind == "ExternalInput":
                if name != partition_name:
                    in_names.append(name)
            elif alloc.kind == "ExternalOutput":
                shape = tuple(alloc.tensor_shape)
                dtype = mybir.dt.np(alloc.dtype)
                out_names.append(name)
                out_avals.append(jax.core.ShapedArray(shape, dtype))
                zero_outs.append(np.zeros(shape, dtype))
        n_params = len(in_names)
        n_outs = len(out_avals)
        in_names.extend(out_names)
        if partition_name is not None:
            in_names.append(partition_name)
        donate = tuple(range(n_params, n_params + n_outs))

        def _body(*args):
            operands = list(args)
            if partition_name is not None:
                operands.append(partition_id_tensor())
            return tuple(
                _bass_exec_p.bind(
                    *operands,
                    out_avals=tuple(out_avals),
                    in_names=tuple(in_names),
                    out_names=tuple(out_names),
                    lowering_input_output_aliases=(),
                    sim_require_finite=True,
                    sim_require_nnan=True,
                    nc=nc,
                )
            )

        sharded = jax.jit(
            shard_map(
                _body,
                mesh=mesh,
                in_specs=(PartitionSpec("core"),) * (n_params + n_outs),
                out_specs=(PartitionSpec("core"),) * len(out_names),
                check_rep=False,
            ),
            donate_argnums=donate,
            keep_unused=True,
        )
        concat_in = [
            jax.device_put(
                np.concatenate(
                    [np.asarray(in_maps[c][nm]) for c in range(n_cores)], 0
                ),
                sh,
            )
            for nm in in_names[:n_params]
        ]
        zfns = [
            jax.jit(
                lambda shp=(n_cores * z.shape[0], *z.shape[1:]), dt=z.dtype: jnp.zeros(
                    shp, dt
                ),
                out_shardings=sh,
            )
            for z in zero_outs
        ]

        def run_once():
            z = [f() for f in zfns]
            jax.block_until_ready(z)
            t0 = time.perf_counter()
            o = sharded(*concat_in, *z)
            jax.block_until_ready(o)
            return time.perf_counter() - t0, o

        return run_once, out_names

    nc1 = build_bitnet_nc(
        M, K, N_shard, N * K, n_cores=n_cores, nsplits=nsplits, reps=1
    )
    run1, out_names = make_runner(nc1)
    t_warm, out_arrs = run1()  # includes NEFF compile+load

    ncR = build_bitnet_nc(
        M, K, N_shard, N * K, n_cores=n_cores, nsplits=nsplits, reps=reps
    )
    runR, _ = make_runner(ncR)
    runR()  # warmup/compile

    t1s, tRs = [], []
    for _ in range(rounds):
        t1s.append(run1()[0])
        tRs.append(runR()[0])
    t1 = min(t1s)
    tR = min(tRs)
    per_exec = (tR - t1) / (reps - 1)
    diag = {"t1_min": t1, "tR_min": tR, "t1s": t1s, "tRs": tRs}

    oi = out_names.index("out")
    glob = np.asarray(out_arrs[oi]).reshape(n_cores, M, N_shard)
    out = np.empty((M, N), dtype=np.float32)
    for c in range(n_cores):
        out[:, c * N_shard : (c + 1) * N_shard] = glob[c]
    return out.reshape(*lead_shape, N), per_exec, diag

